# revision 22
# baseline (speedup 1.0000x reference)
"""Trainium2 Bass kernel for nn_AttentionBlock (RMSNorm + QKV + causal
attention with softmax over the QUERY axis + output projection).

Sharding: data-parallel over batch. B=8 -> one batch element per NeuronCore,
no collectives. Weights are re-laid-out on the host (de-interleave the
(h, dh, 3) QKV packing, transpose so the contraction dim d lands on SBUF
partitions, and pre-tile so every DMA is contiguous).

Device dataflow per core (S=1024, D=1024, H=16, Dh=64):
  1. x natural [s,d] tiles -> sum(x^2) via ACT Square+accum -> rsqrt scale ->
     normed (in-place) -> PE-transpose to normT [d, s].
  2. qkT[f, s] = Wqk^T.T @ normT   (f grouped (c=q/k, head, dh); a 128-row
     f-tile holds a HEAD PAIR: head 2t on partitions 0-63, 2t+1 on 64-127)
     V[s, f] = normT.T @ Wv^T      (natural layout, used as attn@V lhsT)
  3. per head pair t, per k-tile ki: scoresT[k, q] = K^T.T @ Q^T with K=64
     (heads a/b use partition ranges 0-63 / 64-127 -> distinct PE row groups,
     run concurrently).  Causal: only q >= 128*ki computed.
  4. diag block gets additive -1e30 triangle mask (softmax over q == row
     softmax in this transposed layout); one ACT Exp per (head, ki)
     PSUM->SBUF with accum_out giving row sums for free.  Normalization is
     folded into V: vsc = V * (1/rowsum) -> z^T[dh, q] = vsc.T @ attnT.
  5. out[s, :] = z.T @ Wo^T accumulated over f tiles, DMA out.

All matmuls run as float32r (fp32 storage; ~2 PE cycles/row + self-loading
weights on TRN2, fp32 PSUM accumulation).  Measured ~4.6e-4 relative error
(absmax-normalized) end to end; ~414 us per core on HW.  fp16/bf16 were
evaluated and rejected: fp16 also runs at 2 cycles/row on this PE, and
bf16's 8-bit mantissa is unacceptable for Q/K because the unscaled scores
(|s| up to ~27) are amplified by exp.
"""

import numpy as np
from contextlib import ExitStack

import concourse.bacc as bacc
import concourse.bass as bass
import concourse.tile as tile
from concourse import mybir
from concourse.bass_utils import run_bass_kernel_spmd

B, S, DM, H, DH = 8, 1024, 1024, 16, 64
P = 128
EPS = 1.1920929e-07
NEG = -1e30
F32 = mybir.dt.float32
F32R = mybir.dt.float32r
NS = S // P      # 8 s-tiles (also k-tiles)
ND = DM // P     # 8 d-chunks
NPAIR = H // 2   # 8 head pairs
QCH = 512        # moving-dim chunk (one PSUM bank of fp32)
NQC = S // QCH   # 2 q chunks

F16 = mybir.dt.float16

# PREC="f32r": every matmul operand is float32r (~4.6e-4 rel err).
# PREC="mixed": projection-side operands in fp16 (~1e-3 rel err, ~6% faster
# on HW -- fp16 turned out to also run at 2 PE cycles/row, so not worth it).
PREC = "f32r"
MMDT = F32R                      # attn tiles / vsc (z matmul operands)
PDT = F16 if PREC == "mixed" else F32R   # projection-side matmul operands


def _mm(ap):
    return ap


def build_program(with_bias=False):
    nc = bacc.Bacc("TRN2", target_bir_lowering=False, debug=False)

    xb = nc.dram_tensor("xb", [S, DM], F32, kind="ExternalInput").ap()
    # (ft, dd, dk, f): ft 0-7 = Q pair tiles, 8-15 = K pair tiles
    wqk = nc.dram_tensor("wqk", [2 * NPAIR, P, ND, P], PDT, kind="ExternalInput").ap()
    wv = nc.dram_tensor("wv", [ND, P, DM], PDT, kind="ExternalInput").ap()
    wo = nc.dram_tensor("wo", [ND, P, DM], PDT, kind="ExternalInput").ap()
    ident = nc.dram_tensor("ident", [P, P], F32, kind="ExternalInput").ap()
    trimask = nc.dram_tensor("trimask", [P, P], F32, kind="ExternalInput").ap()
    bqk = bv = None
    if with_bias:
        bqk = nc.dram_tensor("bqk", [P, 2 * NPAIR], F32, kind="ExternalInput").ap()
        bv = nc.dram_tensor("bv", [1, DM], PDT, kind="ExternalInput").ap()
    out = nc.dram_tensor("out", [S, DM], F32, kind="ExternalOutput").ap()

    with tile.TileContext(nc) as tc:
        with ExitStack() as ctx:
            _build_body(ctx, tc, xb, wqk, wv, wo, ident, trimask, bqk, bv, out)
    nc.compile()
    return nc


def _build_body(ctx, tc, xb, wqk, wv, wo, ident, trimask, bqk, bv, out):
    nc = tc.nc
    AF = mybir.ActivationFunctionType
    ALU = mybir.AluOpType

    # the (unused in practice) bias path spends a few KB/partition on bias
    # tiles; shrink two pipeline pools there to stay inside SBUF
    nb = 1 if bqk is not None else 2
    singles = ctx.enter_context(tc.tile_pool(name="singles", bufs=1))
    big8 = ctx.enter_context(tc.tile_pool(name="big8", bufs=1))    # x-nat then V
    scr = ctx.enter_context(tc.tile_pool(name="scr", bufs=nb))     # out tiles
    nt = ctx.enter_context(tc.tile_pool(name="nt", bufs=1))        # normT
    w8 = ctx.enter_context(tc.tile_pool(name="w8", bufs=1))        # wv then wo
    wqks = ctx.enter_context(tc.tile_pool(name="wqks", bufs=nb))   # wqk stream
    qkp = ctx.enter_context(tc.tile_pool(name="qkp", bufs=2))      # qt/kt tiles
    attnp = ctx.enter_context(tc.tile_pool(name="attnp", bufs=2))  # exp(scores)
    ztp = ctx.enter_context(tc.tile_pool(name="ztp", bufs=1))      # z transposed
    vscp = ctx.enter_context(tc.tile_pool(name="vscp", bufs=2))    # scaled V
    sm = ctx.enter_context(tc.tile_pool(name="sm", bufs=3))        # small stats
    ps = ctx.enter_context(tc.tile_pool(name="ps", bufs=1, space="PSUM"))

    ident_sb = singles.tile([P, P], F32, tag="ident")
    nc.sync.dma_start(out=ident_sb, in_=ident)
    eps_sb = singles.tile([P, 1], F32, tag="eps")
    nc.vector.memset(eps_sb, EPS)
    tri_sb = singles.tile([P, P], F32, tag="tri")
    nc.sync.dma_start(out=tri_sb, in_=trimask)
    bqk_sb = bv_sb = ones_sb = None
    if bqk is not None:
        bqk_sb = singles.tile([P, 2 * NPAIR], F32, tag="bqk")
        nc.sync.dma_start(out=bqk_sb, in_=bqk)
        bv_sb = singles.tile([1, DM], PDT, tag="bv")
        nc.sync.dma_start(out=bv_sb, in_=bv)
        ones_sb = singles.tile([1, P], PDT, tag="ones")
        nc.vector.memset(ones_sb, 1.0)

    # ---------------- Phase A: RMSNorm + transpose ----------------
    x_ts = []
    for st in range(NS):
        x_t = big8.tile([P, DM], F32, tag=f"b{st}", name=f"xnat{st}")
        nc.sync.dma_start(out=x_t, in_=xb[st * P:(st + 1) * P, :])
        x_ts.append(x_t)
    normT = [nt.tile([P, S], PDT, tag=f"nt{dk}", name=f"normT{dk}")
             for dk in range(ND)]
    for st in range(NS):
        x_t = x_ts[st]
        # squares go to PSUM scratch (never read; only accum_out matters),
        # two halves so the scratch fits a single-bank 'sc' slot
        ssum2 = sm.tile([P, 2], F32, tag="ssum", name=f"ssum{st}")
        for h in range(2):
            sq_ps = ps.tile([P, QCH], F32, tag="sc", bufs=4, name=f"sqps{st}_{h}")
            nc.scalar.activation(out=sq_ps, in_=x_t[:, h * QCH:(h + 1) * QCH],
                                 func=AF.Square, accum_out=ssum2[:, h:h + 1])
        ssum = sm.tile([P, 1], F32, tag="ssA", name=f"ssA{st}")
        nc.vector.tensor_add(out=ssum, in0=ssum2[:, 0:1], in1=ssum2[:, 1:2])
        rs_t = sm.tile([P, 1], F32, tag="rs", name=f"rs{st}")
        # rs = sqrt(mean(x^2) + eps) then reciprocal
        nc.scalar.activation(out=rs_t, in_=ssum, func=AF.Sqrt,
                             bias=eps_sb, scale=1.0 / DM)
        nc.vector.reciprocal(out=rs_t, in_=rs_t)
        nc.vector.tensor_scalar_mul(out=x_t, in0=x_t, scalar1=rs_t)
        for dk in range(ND):
            tp_ps = ps.tile([P, P], F32, tag="mm", bufs=2, name=f"tp{st}_{dk}")
            nc.tensor.transpose(tp_ps, x_t[:, dk * P:(dk + 1) * P], ident_sb)
            nc.vector.tensor_copy(out=normT[dk][:, st * P:(st + 1) * P], in_=tp_ps)

    # ---------------- QK projection helper ----------------
    qt_tiles = {}
    kt_tiles = {}

    def emit_qk_pair(t):
        """qkT tiles for pair t: Q (ft=t) and K (ft=8+t)."""
        for which, ft in (("qt", t), ("kt", NPAIR + t)):
            w_t = wqks.tile([P, ND, P], PDT, tag="wqk", name=f"wqk{ft}")
            nc.sync.dma_start(out=w_t, in_=wqk[ft])
            dst = qkp.tile([P, S], PDT, tag=which, name=f"{which}{t}")
            if which == "qt":
                qt_tiles[t] = dst
            else:
                kt_tiles[t] = dst
            for sc in range(NQC):
                mm_ps = ps.tile([P, QCH], F32, tag="mm", bufs=2, name=f"qkps{ft}_{sc}")
                for dk in range(ND):
                    nc.tensor.matmul(
                        mm_ps,
                        _mm(w_t[:, dk, :]),
                        _mm(normT[dk][:, sc * QCH:(sc + 1) * QCH]),
                        start=(dk == 0), stop=(dk == ND - 1),
                    )
                if bqk_sb is not None:
                    nc.vector.tensor_scalar_add(
                        out=dst[:, sc * QCH:(sc + 1) * QCH], in0=mm_ps,
                        scalar1=bqk_sb[:, ft:ft + 1])
                else:
                    nc.vector.tensor_copy(
                        out=dst[:, sc * QCH:(sc + 1) * QCH], in_=mm_ps)

    # ---------------- Phase B: QK pairs 0,1 then V ----------------
    emit_qk_pair(0)
    emit_qk_pair(1)

    wv_sb = []
    for dk in range(ND):
        w_t = w8.tile([P, DM], PDT, tag=f"w{dk}", name=f"wv{dk}")
        nc.sync.dma_start(out=w_t, in_=wv[dk])
        wv_sb.append(w_t)
    vs = []
    for st in range(NS):
        v_t = big8.tile([P, DM], PDT, tag=f"b{st}", name=f"vs{st}")
        vs.append(v_t)
        for fvc in range(NQC):
            mm_ps = ps.tile([P, QCH], F32, tag="mm", bufs=2, name=f"vps{st}_{fvc}")
            for dk in range(ND):
                nc.tensor.matmul(
                    mm_ps,
                    _mm(normT[dk][:, st * P:(st + 1) * P]),
                    _mm(wv_sb[dk][:, fvc * QCH:(fvc + 1) * QCH]),
                    start=(dk == 0),
                    stop=(dk == ND - 1 and bv_sb is None),
                )
            if bv_sb is not None:
                nc.tensor.matmul(
                    mm_ps, _mm(ones_sb),
                    _mm(bv_sb[:, fvc * QCH:(fvc + 1) * QCH]),
                    start=False, stop=True,
                )
            nc.vector.tensor_copy(out=v_t[:, fvc * QCH:(fvc + 1) * QCH], in_=mm_ps)

    # prefetch Wo early: the w8 slots free up as soon as V consumed wv
    wo_sb = []
    for fk in range(ND):
        w_t = w8.tile([P, DM], PDT, tag=f"w{fk}", name=f"wo{fk}")
        nc.sync.dma_start(out=w_t, in_=wo[fk])
        wo_sb.append(w_t)

    # ---------------- Phase C/D: attention per head pair ----------------
    zT = []
    for t in range(NPAIR):
        if t + 2 < NPAIR:
            emit_qk_pair(t + 2)
        qt, kt = qt_tiles.pop(t), kt_tiles.pop(t)

        z_t = ztp.tile([P, S], PDT, tag=f"zt{t}", name=f"zT{t}")
        zT.append(z_t)
        attn = {}   # (head_local, ki) -> sbuf tile [P, S - ki*P]
        vsc_d = {}  # ki -> scaled V slice [P, 128] for this pair

        def emit_z_chunk(qc, t=t, z_t=z_t, attn=attn, vsc_d=vsc_d):
            kis = [ki for ki in range(NS) if ki * P < (qc + 1) * QCH]
            for hl in (0, 1):
                z_ps = ps.tile([DH, QCH], F32, tag="z", bufs=2, name=f"zps{t}_{hl}_{qc}")
                for i, ki in enumerate(kis):
                    q0 = max(qc * QCH, ki * P)
                    a_t = attn[(hl, ki)]
                    nc.tensor.matmul(
                        z_ps[:, q0 - qc * QCH:QCH],
                        _mm(vsc_d[ki][:, hl * DH:(hl + 1) * DH]),
                        _mm(a_t[:, q0 - ki * P:(qc + 1) * QCH - ki * P]),
                        start=(i == 0), stop=(i == len(kis) - 1),
                    )
                nc.vector.tensor_copy(
                    out=z_t[hl * DH:(hl + 1) * DH, qc * QCH:(qc + 1) * QCH],
                    in_=z_ps)

        for ki in range(NS):
            width = S - ki * P
            diag_qc = 0 if ki < 4 else 1
            sc_ps = {}
            for hl, prange in ((0, slice(0, DH)), (1, slice(DH, P))):
                a_t = attnp.tile([P, width], MMDT, tag=f"at{ki}",
                                 name=f"attn{t}_{hl}_{ki}")
                attn[(hl, ki)] = a_t
                for qc in range(NQC):
                    if qc == 0 and ki >= 4:
                        continue
                    q0 = max(qc * QCH, ki * P)
                    q1 = (qc + 1) * QCH
                    s_ps = ps.tile([P, QCH], F32, tag="sc", bufs=4,
                                   name=f"sps{t}_{hl}_{ki}_{qc}")
                    sc_ps[(hl, qc)] = s_ps
                    nc.tensor.matmul(
                        s_ps[:, q0 - qc * QCH:QCH],
                        _mm(kt[prange, ki * P:(ki + 1) * P]),
                        _mm(qt[prange, q0:q1]),
                        start=True, stop=True,
                    )
                    if qc == diag_qc:
                        dlo = ki * P - qc * QCH
                        nc.vector.tensor_add(
                            out=s_ps[:, dlo:dlo + P],
                            in0=s_ps[:, dlo:dlo + P], in1=tri_sb)
            # piecewise exp (one per PSUM bank) with fused row-sum pieces;
            # each bank frees as soon as its own exp drains it
            rsp = sm.tile([P, 4], F32, tag="rsp", name=f"rsp{t}_{ki}")
            for hl in (0, 1):
                a_t = attn[(hl, ki)]
                if ki < 4:
                    nc.scalar.activation(
                        out=a_t[:, 0:QCH - ki * P],
                        in_=sc_ps[(hl, 0)][:, ki * P:QCH],
                        func=AF.Exp, accum_out=rsp[:, hl:hl + 1])
                    nc.scalar.activation(
                        out=a_t[:, QCH - ki * P:width],
                        in_=sc_ps[(hl, 1)],
                        func=AF.Exp, accum_out=rsp[:, 2 + hl:3 + hl])
                else:
                    nc.scalar.activation(
                        out=a_t[:, 0:width],
                        in_=sc_ps[(hl, 1)][:, ki * P - QCH:QCH],
                        func=AF.Exp, accum_out=rsp[:, 2 + hl:3 + hl])
            ri = sm.tile([P, 2], F32, tag="ri", name=f"ri{t}_{ki}")
            if ki < 4:
                nc.vector.tensor_add(out=ri, in0=rsp[:, 0:2], in1=rsp[:, 2:4])
                nc.vector.reciprocal(out=ri, in_=ri)
            else:
                nc.vector.reciprocal(out=ri, in_=rsp[:, 2:4])
            # scaled V slice for this (pair, ki): [128k, 2*64]
            vsc = vscp.tile([P, P], MMDT, tag=f"vsc{ki}", name=f"vsc{t}_{ki}")
            vsc_d[ki] = vsc
            ri_b = bass.AP(tensor=ri.tensor, offset=ri.offset,
                           ap=[list(ri.ap[0]), list(ri.ap[1]), [0, DH]])
            nc.vector.tensor_tensor(
                out=vsc.rearrange("p (h d) -> p h d", h=2),
                in0=vs[ki][:, t * P:(t + 1) * P].rearrange("p (h d) -> p h d", h=2),
                in1=ri_b, op=ALU.mult)
            if ki == 3:
                emit_z_chunk(0)
        emit_z_chunk(1)

    # ---------------- Phase E: output projection ----------------
    for st in range(NS):
        o_t = scr.tile([P, DM], F32, tag="osb", name=f"osb{st}")
        for dmc in range(NQC):
            mm_ps = ps.tile([P, QCH], F32, tag="mm", bufs=2, name=f"ops{st}_{dmc}")
            for fk in range(ND):
                nc.tensor.matmul(
                    mm_ps,
                    _mm(zT[fk][:, st * P:(st + 1) * P]),
                    _mm(wo_sb[fk][:, dmc * QCH:(dmc + 1) * QCH]),
                    start=(fk == 0), stop=(fk == ND - 1),
                )
            nc.vector.tensor_copy(out=o_t[:, dmc * QCH:(dmc + 1) * QCH], in_=mm_ps)
        nc.sync.dma_start(out=out[st * P:(st + 1) * P, :], in_=o_t)


NP_PDT = np.float16 if PREC == "mixed" else np.float32


def prep_inputs(x, W_qkv, b_qkv):
    """Host-side re-layout of inputs (weights de-interleave/transpose/tile)."""
    x = np.ascontiguousarray(np.asarray(x, np.float32))
    W = np.asarray(W_qkv, np.float32).reshape(H, DH, 3, DM)
    Wq = W[:, :, 0, :].reshape(H * DH, DM)
    Wk = W[:, :, 1, :].reshape(H * DH, DM)
    Wv = W[:, :, 2, :].reshape(H * DH, DM)
    WqkT = np.ascontiguousarray(np.concatenate([Wq, Wk], 0).T)   # [DM, 2048]
    wqk_host = np.ascontiguousarray(
        WqkT.reshape(ND, P, 2 * NPAIR, P).transpose(2, 1, 0, 3)).astype(NP_PDT)
    wv_host = np.ascontiguousarray(Wv.T).reshape(ND, P, DM).astype(NP_PDT)
    ident = np.eye(P, dtype=np.float32)
    idx = np.arange(P)
    trimask = np.where(idx[None, :] >= idx[:, None], 0.0, NEG).astype(np.float32)

    b = np.asarray(b_qkv, np.float32).reshape(H, DH, 3)
    bq = b[:, :, 0].reshape(H * DH)
    bk = b[:, :, 1].reshape(H * DH)
    bvv = b[:, :, 2].reshape(H * DH)
    bqk_host = np.ascontiguousarray(
        np.concatenate([bq, bk]).reshape(2 * NPAIR, P).T)         # [P, 16]
    return x, wqk_host, wv_host, ident, trimask, bqk_host, bvv


_prog_cache = {}


def kernel(x, W_qkv, b_qkv, W_o, b_o, trace=False):
    x, wqk_host, wv_host, ident, trimask, bqk_host, bvv = prep_inputs(
        x, W_qkv, b_qkv)
    wo_host = np.ascontiguousarray(np.asarray(W_o, np.float32).T).reshape(ND, P, DM).astype(NP_PDT)
    with_bias = bool(np.any(np.asarray(b_qkv)))
    key = (PREC, with_bias)
    if key not in _prog_cache:
        _prog_cache[key] = build_program(with_bias=with_bias)
    nc = _prog_cache[key]

    in_maps = []
    for bi in range(B):
        m = {
            "xb": x[bi], "wqk": wqk_host, "wv": wv_host, "wo": wo_host,
            "ident": ident, "trimask": trimask,
        }
        if with_bias:
            m["bqk"] = bqk_host
            m["bv"] = bvv.reshape(1, DM).astype(NP_PDT)
        in_maps.append(m)

    res = run_bass_kernel_spmd(nc, in_maps, core_ids=list(range(B)), trace=trace)
    out = np.stack([res.results[bi]["out"] for bi in range(B)]).astype(np.float32)
    out += np.asarray(b_o, np.float32)[None, None, :]
    if trace:
        kernel.last_results = res
    return out


# revision 24
# speedup vs baseline: 1.1875x; 1.1875x over previous
"""Trainium2 Bass kernel for nn_AttentionBlock (RMSNorm + QKV + causal
attention with softmax over the QUERY axis + output projection).

Sharding: data-parallel over batch. B=8 -> one batch element per NeuronCore,
no collectives. Weights are re-laid-out on the host (de-interleave the
(h, dh, 3) QKV packing, transpose so the contraction dim d lands on SBUF
partitions, and pre-tile so every DMA is contiguous).

Device dataflow per core (S=1024, D=1024, H=16, Dh=64):
  1. x natural [s,d] tiles -> sum(x^2) via ACT Square+accum -> rsqrt scale ->
     normed (in-place) -> PE-transpose to normT [d, s].
  2. qkT[f, s] = Wqk^T.T @ normT   (f grouped (c=q/k, head, dh); a 128-row
     f-tile holds a HEAD PAIR: head 2t on partitions 0-63, 2t+1 on 64-127)
     V[s, f] = normT.T @ Wv^T      (natural layout, used as attn@V lhsT)
  3. per head pair t, per k-tile ki: scoresT[k, q] = K^T.T @ Q^T with K=64
     (heads a/b use partition ranges 0-63 / 64-127 -> distinct PE row groups,
     run concurrently).  Causal: only q >= 128*ki computed.
  4. diag block gets additive -1e30 triangle mask (softmax over q == row
     softmax in this transposed layout); one ACT Exp per (head, ki)
     PSUM->SBUF with accum_out giving row sums for free.  Normalization is
     folded into V: vsc = V * (1/rowsum) -> z^T[dh, q] = vsc.T @ attnT.
  5. out[s, :] = z.T @ Wo^T accumulated over f tiles, DMA out.

All matmuls run as float32r (fp32 storage; ~2 PE cycles/row + self-loading
weights on TRN2, fp32 PSUM accumulation).  Measured ~4.6e-4 relative error
(absmax-normalized) end to end; ~414 us per core on HW.  fp16/bf16 were
evaluated and rejected: fp16 also runs at 2 cycles/row on this PE, and
bf16's 8-bit mantissa is unacceptable for Q/K because the unscaled scores
(|s| up to ~27) are amplified by exp.
"""

import numpy as np
from contextlib import ExitStack

import concourse.bacc as bacc
import concourse.bass as bass
import concourse.tile as tile
from concourse import mybir
from concourse.bass_utils import run_bass_kernel_spmd

B, S, DM, H, DH = 8, 1024, 1024, 16, 64
P = 128
EPS = 1.1920929e-07
NEG = -1e30
F32 = mybir.dt.float32
F32R = mybir.dt.float32r
NS = S // P      # 8 s-tiles (also k-tiles)
ND = DM // P     # 8 d-chunks
NPAIR = H // 2   # 8 head pairs
QCH = 512        # moving-dim chunk (one PSUM bank of fp32)
NQC = S // QCH   # 2 q chunks

F16 = mybir.dt.float16

# PREC="f32r": every matmul operand is float32r (~4.6e-4 rel err).
# PREC="mixed": projection-side operands in fp16 (~1e-3 rel err, ~6% faster
# on HW -- fp16 turned out to also run at 2 PE cycles/row, so not worth it).
PREC = "f32r"
MMDT = F32R                      # attn tiles / vsc (z matmul operands)
PDT = F16 if PREC == "mixed" else F32R   # projection-side matmul operands


def _mm(ap):
    return ap


def build_program(with_bias=False):
    nc = bacc.Bacc("TRN2", target_bir_lowering=False, debug=False)

    xb = nc.dram_tensor("xb", [S, DM], F32, kind="ExternalInput").ap()
    # (ft, dd, dk, f): ft 0-7 = Q pair tiles, 8-15 = K pair tiles
    wqk = nc.dram_tensor("wqk", [2 * NPAIR, P, ND, P], PDT, kind="ExternalInput").ap()
    wv = nc.dram_tensor("wv", [ND, P, DM], PDT, kind="ExternalInput").ap()
    wo = nc.dram_tensor("wo", [ND, P, DM], PDT, kind="ExternalInput").ap()
    ident = nc.dram_tensor("ident", [P, P], F32, kind="ExternalInput").ap()
    trimask = nc.dram_tensor("trimask", [P, P], F32, kind="ExternalInput").ap()
    bqk = bv = None
    if with_bias:
        bqk = nc.dram_tensor("bqk", [P, 2 * NPAIR], F32, kind="ExternalInput").ap()
        bv = nc.dram_tensor("bv", [1, DM], PDT, kind="ExternalInput").ap()
    out = nc.dram_tensor("out", [S, DM], F32, kind="ExternalOutput").ap()

    with tile.TileContext(nc) as tc:
        with ExitStack() as ctx:
            _build_body(ctx, tc, xb, wqk, wv, wo, ident, trimask, bqk, bv, out)
    nc.compile()
    return nc


def _build_body(ctx, tc, xb, wqk, wv, wo, ident, trimask, bqk, bv, out):
    nc = tc.nc
    AF = mybir.ActivationFunctionType
    ALU = mybir.AluOpType

    # the (unused in practice) bias path spends a few KB/partition on bias
    # tiles; shrink two pipeline pools there to stay inside SBUF
    nb = 1 if bqk is not None else 2
    singles = ctx.enter_context(tc.tile_pool(name="singles", bufs=1))
    big8 = ctx.enter_context(tc.tile_pool(name="big8", bufs=1))    # x-nat then V
    scr = ctx.enter_context(tc.tile_pool(name="scr", bufs=nb))     # out tiles
    nt = ctx.enter_context(tc.tile_pool(name="nt", bufs=1))        # normT
    w8 = ctx.enter_context(tc.tile_pool(name="w8", bufs=1))        # wv then wo
    wqks = ctx.enter_context(tc.tile_pool(name="wqks", bufs=nb))   # wqk stream
    qkp = ctx.enter_context(tc.tile_pool(name="qkp", bufs=2))      # qt/kt tiles
    attnp = ctx.enter_context(tc.tile_pool(name="attnp", bufs=2))  # exp(scores)
    ztp = ctx.enter_context(tc.tile_pool(name="ztp", bufs=1))      # z transposed
    vscp = ctx.enter_context(tc.tile_pool(name="vscp", bufs=2))    # scaled V
    sm = ctx.enter_context(tc.tile_pool(name="sm", bufs=3))        # small stats
    ps = ctx.enter_context(tc.tile_pool(name="ps", bufs=1, space="PSUM"))

    ident_sb = singles.tile([P, P], F32, tag="ident")
    nc.sync.dma_start(out=ident_sb, in_=ident)
    eps_sb = singles.tile([P, 1], F32, tag="eps")
    nc.vector.memset(eps_sb, EPS)
    tri_sb = singles.tile([P, P], F32, tag="tri")
    nc.sync.dma_start(out=tri_sb, in_=trimask)
    bqk_sb = bv_sb = ones_sb = None
    if bqk is not None:
        bqk_sb = singles.tile([P, 2 * NPAIR], F32, tag="bqk")
        nc.sync.dma_start(out=bqk_sb, in_=bqk)
        bv_sb = singles.tile([1, DM], PDT, tag="bv")
        nc.sync.dma_start(out=bv_sb, in_=bv)
        ones_sb = singles.tile([1, P], PDT, tag="ones")
        nc.vector.memset(ones_sb, 1.0)

    # ---------------- Phase A: RMSNorm + transpose ----------------
    x_ts = []
    for st in range(NS):
        x_t = big8.tile([P, DM], F32, tag=f"b{st}", name=f"xnat{st}")
        nc.sync.dma_start(out=x_t, in_=xb[st * P:(st + 1) * P, :])
        x_ts.append(x_t)
    normT = [nt.tile([P, S], PDT, tag=f"nt{dk}", name=f"normT{dk}")
             for dk in range(ND)]
    for st in range(NS):
        x_t = x_ts[st]
        # squares go to PSUM scratch (never read; only accum_out matters),
        # two halves so the scratch fits a single-bank 'sc' slot
        ssum2 = sm.tile([P, 2], F32, tag="ssum", name=f"ssum{st}")
        for h in range(2):
            sq_ps = ps.tile([P, QCH], F32, tag="sc", bufs=4, name=f"sqps{st}_{h}")
            nc.scalar.activation(out=sq_ps, in_=x_t[:, h * QCH:(h + 1) * QCH],
                                 func=AF.Square, accum_out=ssum2[:, h:h + 1])
        ssum = sm.tile([P, 1], F32, tag="ssA", name=f"ssA{st}")
        nc.vector.tensor_add(out=ssum, in0=ssum2[:, 0:1], in1=ssum2[:, 1:2])
        rs_t = sm.tile([P, 1], F32, tag="rs", name=f"rs{st}")
        # rs = sqrt(mean(x^2) + eps) then reciprocal
        nc.scalar.activation(out=rs_t, in_=ssum, func=AF.Sqrt,
                             bias=eps_sb, scale=1.0 / DM)
        nc.vector.reciprocal(out=rs_t, in_=rs_t)
        nc.vector.tensor_scalar_mul(out=x_t, in0=x_t, scalar1=rs_t)
        for dk in range(ND):
            tp_ps = ps.tile([P, P], F32, tag="mm", bufs=2, name=f"tp{st}_{dk}")
            nc.tensor.transpose(tp_ps, x_t[:, dk * P:(dk + 1) * P], ident_sb)
            nc.vector.tensor_copy(out=normT[dk][:, st * P:(st + 1) * P], in_=tp_ps)

    # ---------------- QK projection helper ----------------
    qt_tiles = {}
    kt_tiles = {}

    def emit_qk_pair(t):
        """qkT tiles for pair t: Q (ft=t) and K (ft=8+t)."""
        for which, ft in (("qt", t), ("kt", NPAIR + t)):
            w_t = wqks.tile([P, ND, P], PDT, tag="wqk", name=f"wqk{ft}")
            nc.sync.dma_start(out=w_t, in_=wqk[ft])
            dst = qkp.tile([P, S], PDT, tag=which, name=f"{which}{t}")
            if which == "qt":
                qt_tiles[t] = dst
            else:
                kt_tiles[t] = dst
            for sc in range(NQC):
                mm_ps = ps.tile([P, QCH], F32, tag="mm", bufs=2, name=f"qkps{ft}_{sc}")
                for dk in range(ND):
                    nc.tensor.matmul(
                        mm_ps,
                        _mm(w_t[:, dk, :]),
                        _mm(normT[dk][:, sc * QCH:(sc + 1) * QCH]),
                        start=(dk == 0), stop=(dk == ND - 1),
                    )
                if bqk_sb is not None:
                    nc.vector.tensor_scalar_add(
                        out=dst[:, sc * QCH:(sc + 1) * QCH], in0=mm_ps,
                        scalar1=bqk_sb[:, ft:ft + 1])
                else:
                    nc.vector.tensor_copy(
                        out=dst[:, sc * QCH:(sc + 1) * QCH], in_=mm_ps)

    # ---------------- Phase B: QK pairs 0,1 then V ----------------
    emit_qk_pair(0)
    emit_qk_pair(1)

    wv_sb = []
    for dk in range(ND):
        w_t = w8.tile([P, DM], PDT, tag=f"w{dk}", name=f"wv{dk}")
        nc.sync.dma_start(out=w_t, in_=wv[dk])
        wv_sb.append(w_t)
    vs = []
    for st in range(NS):
        v_t = big8.tile([P, DM], PDT, tag=f"b{st}", name=f"vs{st}")
        vs.append(v_t)
        for fvc in range(NQC):
            mm_ps = ps.tile([P, QCH], F32, tag="mm", bufs=2, name=f"vps{st}_{fvc}")
            for dk in range(ND):
                nc.tensor.matmul(
                    mm_ps,
                    _mm(normT[dk][:, st * P:(st + 1) * P]),
                    _mm(wv_sb[dk][:, fvc * QCH:(fvc + 1) * QCH]),
                    start=(dk == 0),
                    stop=(dk == ND - 1 and bv_sb is None),
                )
            if bv_sb is not None:
                nc.tensor.matmul(
                    mm_ps, _mm(ones_sb),
                    _mm(bv_sb[:, fvc * QCH:(fvc + 1) * QCH]),
                    start=False, stop=True,
                )
            nc.vector.tensor_copy(out=v_t[:, fvc * QCH:(fvc + 1) * QCH], in_=mm_ps)

    # prefetch Wo early: the w8 slots free up as soon as V consumed wv
    wo_sb = []
    for fk in range(ND):
        w_t = w8.tile([P, DM], PDT, tag=f"w{fk}", name=f"wo{fk}")
        nc.sync.dma_start(out=w_t, in_=wo[fk])
        wo_sb.append(w_t)

    # ---------------- Phase C/D: attention per head pair ----------------
    zT = []
    for t in range(NPAIR):
        if t + 2 < NPAIR:
            emit_qk_pair(t + 2)
        qt, kt = qt_tiles.pop(t), kt_tiles.pop(t)

        z_t = ztp.tile([P, S], PDT, tag=f"zt{t}", name=f"zT{t}")
        zT.append(z_t)
        attn = {}   # (head_local, ki) -> sbuf tile [P, S - ki*P]
        vsc_d = {}  # ki -> scaled V slice [P, 128] for this pair

        def emit_z_chunk(qc, t=t, z_t=z_t, attn=attn, vsc_d=vsc_d):
            kis = [ki for ki in range(NS) if ki * P < (qc + 1) * QCH]
            for hl in (0, 1):
                z_ps = ps.tile([DH, QCH], F32, tag="z", bufs=2, name=f"zps{t}_{hl}_{qc}")
                for i, ki in enumerate(kis):
                    q0 = max(qc * QCH, ki * P)
                    a_t = attn[(hl, ki)]
                    nc.tensor.matmul(
                        z_ps[:, q0 - qc * QCH:QCH],
                        _mm(vsc_d[ki][:, hl * DH:(hl + 1) * DH]),
                        _mm(a_t[:, q0 - ki * P:(qc + 1) * QCH - ki * P]),
                        start=(i == 0), stop=(i == len(kis) - 1),
                    )
                nc.vector.tensor_copy(
                    out=z_t[hl * DH:(hl + 1) * DH, qc * QCH:(qc + 1) * QCH],
                    in_=z_ps)

        for ki in range(NS):
            width = S - ki * P
            diag_qc = 0 if ki < 4 else 1
            sc_ps = {}
            for hl, prange in ((0, slice(0, DH)), (1, slice(DH, P))):
                a_t = attnp.tile([P, width], MMDT, tag=f"at{ki}",
                                 name=f"attn{t}_{hl}_{ki}")
                attn[(hl, ki)] = a_t
                for qc in range(NQC):
                    if qc == 0 and ki >= 4:
                        continue
                    q0 = max(qc * QCH, ki * P)
                    q1 = (qc + 1) * QCH
                    s_ps = ps.tile([P, QCH], F32, tag="sc", bufs=4,
                                   name=f"sps{t}_{hl}_{ki}_{qc}")
                    sc_ps[(hl, qc)] = s_ps
                    nc.tensor.matmul(
                        s_ps[:, q0 - qc * QCH:QCH],
                        _mm(kt[prange, ki * P:(ki + 1) * P]),
                        _mm(qt[prange, q0:q1]),
                        start=True, stop=True,
                    )
                    if qc == diag_qc:
                        dlo = ki * P - qc * QCH
                        nc.vector.tensor_add(
                            out=s_ps[:, dlo:dlo + P],
                            in0=s_ps[:, dlo:dlo + P], in1=tri_sb)
            # piecewise exp (one per PSUM bank) with fused row-sum pieces;
            # each bank frees as soon as its own exp drains it
            rsp = sm.tile([P, 4], F32, tag="rsp", name=f"rsp{t}_{ki}")
            for hl in (0, 1):
                a_t = attn[(hl, ki)]
                if ki < 4:
                    nc.scalar.activation(
                        out=a_t[:, 0:QCH - ki * P],
                        in_=sc_ps[(hl, 0)][:, ki * P:QCH],
                        func=AF.Exp, accum_out=rsp[:, hl:hl + 1])
                    nc.scalar.activation(
                        out=a_t[:, QCH - ki * P:width],
                        in_=sc_ps[(hl, 1)],
                        func=AF.Exp, accum_out=rsp[:, 2 + hl:3 + hl])
                else:
                    nc.scalar.activation(
                        out=a_t[:, 0:width],
                        in_=sc_ps[(hl, 1)][:, ki * P - QCH:QCH],
                        func=AF.Exp, accum_out=rsp[:, 2 + hl:3 + hl])
            ri = sm.tile([P, 2], F32, tag="ri", name=f"ri{t}_{ki}")
            if ki < 4:
                nc.vector.tensor_add(out=ri, in0=rsp[:, 0:2], in1=rsp[:, 2:4])
                nc.vector.reciprocal(out=ri, in_=ri)
            else:
                nc.vector.reciprocal(out=ri, in_=rsp[:, 2:4])
            # scaled V slice for this (pair, ki): [128k, 2*64]
            vsc = vscp.tile([P, P], MMDT, tag=f"vsc{ki}", name=f"vsc{t}_{ki}")
            vsc_d[ki] = vsc
            ri_b = bass.AP(tensor=ri.tensor, offset=ri.offset,
                           ap=[list(ri.ap[0]), list(ri.ap[1]), [0, DH]])
            nc.vector.tensor_tensor(
                out=vsc.rearrange("p (h d) -> p h d", h=2),
                in0=vs[ki][:, t * P:(t + 1) * P].rearrange("p (h d) -> p h d", h=2),
                in1=ri_b, op=ALU.mult)
            if ki == 3:
                emit_z_chunk(0)
        emit_z_chunk(1)

    # ---------------- Phase E: output projection ----------------
    for st in range(NS):
        o_t = scr.tile([P, DM], F32, tag="osb", name=f"osb{st}")
        for dmc in range(NQC):
            mm_ps = ps.tile([P, QCH], F32, tag="mm", bufs=2, name=f"ops{st}_{dmc}")
            for fk in range(ND):
                nc.tensor.matmul(
                    mm_ps,
                    _mm(zT[fk][:, st * P:(st + 1) * P]),
                    _mm(wo_sb[fk][:, dmc * QCH:(dmc + 1) * QCH]),
                    start=(fk == 0), stop=(fk == ND - 1),
                )
            nc.vector.tensor_copy(out=o_t[:, dmc * QCH:(dmc + 1) * QCH], in_=mm_ps)
        nc.sync.dma_start(out=out[st * P:(st + 1) * P, :], in_=o_t)


NP_PDT = np.float16 if PREC == "mixed" else np.float32


def prep_inputs(x, W_qkv, b_qkv):
    """Host-side re-layout of inputs (weights de-interleave/transpose/tile)."""
    x = np.ascontiguousarray(np.asarray(x, np.float32))
    W = np.asarray(W_qkv, np.float32).reshape(H, DH, 3, DM)
    Wq = W[:, :, 0, :].reshape(H * DH, DM)
    Wk = W[:, :, 1, :].reshape(H * DH, DM)
    Wv = W[:, :, 2, :].reshape(H * DH, DM)
    WqkT = np.ascontiguousarray(np.concatenate([Wq, Wk], 0).T)   # [DM, 2048]
    wqk_host = np.ascontiguousarray(
        WqkT.reshape(ND, P, 2 * NPAIR, P).transpose(2, 1, 0, 3)).astype(NP_PDT)
    wv_host = np.ascontiguousarray(Wv.T).reshape(ND, P, DM).astype(NP_PDT)
    ident = np.eye(P, dtype=np.float32)
    idx = np.arange(P)
    trimask = np.where(idx[None, :] >= idx[:, None], 0.0, NEG).astype(np.float32)

    b = np.asarray(b_qkv, np.float32).reshape(H, DH, 3)
    bq = b[:, :, 0].reshape(H * DH)
    bk = b[:, :, 1].reshape(H * DH)
    bvv = b[:, :, 2].reshape(H * DH)
    bqk_host = np.ascontiguousarray(
        np.concatenate([bq, bk]).reshape(2 * NPAIR, P).T)         # [P, 16]
    return x, wqk_host, wv_host, ident, trimask, bqk_host, bvv


_prog_cache = {}


def kernel(x, W_qkv, b_qkv, W_o, b_o, trace=False):
    x, wqk_host, wv_host, ident, trimask, bqk_host, bvv = prep_inputs(
        x, W_qkv, b_qkv)
    wo_host = np.ascontiguousarray(np.asarray(W_o, np.float32).T).reshape(ND, P, DM).astype(NP_PDT)
    with_bias = bool(np.any(np.asarray(b_qkv)))
    key = (PREC, with_bias)
    if key not in _prog_cache:
        _prog_cache[key] = build_program(with_bias=with_bias)
    nc = _prog_cache[key]

    in_maps = []
    for bi in range(B):
        m = {
            "xb": x[bi], "wqk": wqk_host, "wv": wv_host, "wo": wo_host,
            "ident": ident, "trimask": trimask,
        }
        if with_bias:
            m["bqk"] = bqk_host
            m["bv"] = bvv.reshape(1, DM).astype(NP_PDT)
        in_maps.append(m)

    res = run_bass_kernel_spmd(nc, in_maps, core_ids=list(range(B)), trace=trace)
    out = np.stack([res.results[bi]["out"] for bi in range(B)]).astype(np.float32)
    out += np.asarray(b_o, np.float32)[None, None, :]
    if trace:
        kernel.last_results = res
    return out
